# revision 1
# baseline (speedup 1.0000x reference)
"""Causal GQA self-attention (B=4, T=2048, C=2048, 16 Q heads / 8 KV heads,
hd=128) as a Bass/Tile SPMD kernel on 8 Trainium2 NeuronCores.

Sharding: core c = (batch b = c//2, head-group g = c%2). Each core handles one
batch and 8 Q heads / 4 KV heads. Wq/Wk/Wv column-sharded on the head dim, Wo
row-sharded; the host sums the two partial Wo products per batch (2-way
all-reduce done on host during the gather).

All on-device tensors live in a transposed [feature, token] layout so every
matmul contraction sits on the partition dim with no on-device transposes:
  qT/kT = [d, t], v = [t, d], scores as S^T = [k, q], output as y^T = [o, t].
Bulk matmuls run in bf16 (fp32 PSUM accumulation; ~4e-3 end-to-end rel err).
The loop is software-pipelined: attention/Wo of block tb-1 interleave with
the projections of block tb so projection matmuls fill PE gaps while the
ScalarE exp stream drains; softmax denominators accumulate on the PE via an
accumulating ones-matmul, reciprocals use the single-op approx DVE path, and
causal masking is a GpSimd memset + one [128,128] triangular multiply.
"""

import sys

import ml_dtypes
import numpy as np

sys.path.insert(0, "/opt/trn_rl_repo")

import concourse.bass as bass  # noqa: E402
import concourse.mybir as mybir  # noqa: E402
import concourse.tile as tile  # noqa: E402
from concourse import bacc  # noqa: E402
from concourse.bass_utils import run_bass_kernel_spmd  # noqa: E402

# Problem shape (hardcoded per contest contract).
B = 4
T = 2048
C = 2048
HD = 128
N_HEAD = 16
N_KV_HEAD = 8
NQH = N_HEAD // 2  # q heads per core (group)
NKV = N_KV_HEAD // 2  # kv heads per core
TB = 512  # token block
NTB = T // TB
NCT = C // 128  # contraction tiles for the projections
SCALE = 1.0 / float(np.sqrt(HD))

F32 = mybir.dt.float32
F32R = mybir.dt.float32r
BF16 = mybir.dt.bfloat16
MULT = mybir.AluOpType.mult
ADD = mybir.AluOpType.add
EXP = mybir.ActivationFunctionType.Exp


def _rope(nc, tmpp, dst, src_psum, cosb, nsinb):
    """dst = src*cos + rot_half(src)*sin, src in [d, t] layout (d partitions).

    rot_half(x)[d] = -x[d+64] for d<64, +x[d-64] for d>=64; the sign lives in
    nsinb so both halves are plain multiplies. nsinb is the sin table rotated
    by 64 partitions (nsinb[64+i] = -sin[i], nsinb[i] = sin[64+i]) so each
    tensor_tensor has equal base partitions on its two SBUF inputs (HW rule).
    """
    t0 = tmpp.tile([HD, TB], F32, tag="t0")
    nc.scalar.copy(t0[:], src_psum[:])
    nc.vector.tensor_mul(dst, t0[:], cosb[:])
    t2 = tmpp.tile([HD, TB], F32, tag="t2")
    nc.vector.tensor_mul(t2[0:64, :], t0[64:128, :], nsinb[64:128, :])
    nc.vector.tensor_mul(t2[64:128, :], t0[0:64, :], nsinb[0:64, :])
    nc.vector.scalar_tensor_tensor(dst, t2[:], 1.0, dst, op0=MULT, op1=ADD)


def build_nc():
    nc = bacc.Bacc("TRN2", target_bir_lowering=False, debug=False, num_devices=8)

    xT = nc.dram_tensor("xT", [C, T], BF16, kind="ExternalInput")
    wqT = nc.dram_tensor("wqT", [C, NQH * HD], BF16, kind="ExternalInput")
    wkT = nc.dram_tensor("wkT", [C, NKV * HD], BF16, kind="ExternalInput")
    wvT = nc.dram_tensor("wvT", [C, NKV * HD], BF16, kind="ExternalInput")
    woT = nc.dram_tensor("woT", [NQH * HD, C], BF16, kind="ExternalInput")
    cosdt = nc.dram_tensor("cosdt", [HD, T], F32, kind="ExternalInput")
    nsindt = nc.dram_tensor("nsindt", [HD, T], F32, kind="ExternalInput")
    masks = nc.dram_tensor("masks", [4, 128, TB], BF16, kind="ExternalInput")
    onescol = nc.dram_tensor("onescol", [128, 1], BF16, kind="ExternalInput")
    onesrow = nc.dram_tensor("onesrow", [1, 128], F32R, kind="ExternalInput")
    yT = nc.dram_tensor("yT", [C, T], F32, kind="ExternalOutput")

    from contextlib import ExitStack

    with ExitStack() as es:
        tc = es.enter_context(tile.TileContext(nc))
        es.enter_context(nc.allow_low_precision("fp32r attention"))
        constp = es.enter_context(tc.tile_pool(name="const", bufs=1))
        strp = es.enter_context(tc.tile_pool(name="stream", bufs=2))
        perp = es.enter_context(tc.tile_pool(name="persist", bufs=1))
        xp = es.enter_context(tc.tile_pool(name="xp", bufs=16))
        wqp = es.enter_context(tc.tile_pool(name="wq", bufs=2))
        wkp = es.enter_context(tc.tile_pool(name="wk", bufs=2))
        wvp = es.enter_context(tc.tile_pool(name="wv", bufs=2))
        wop = es.enter_context(tc.tile_pool(name="wo", bufs=3))
        qp = es.enter_context(tc.tile_pool(name="qt", bufs=16))
        outp = es.enter_context(tc.tile_pool(name="ot", bufs=8))
        tmpp = es.enter_context(tc.tile_pool(name="tmp", bufs=2))
        expp = es.enter_context(tc.tile_pool(name="exps", bufs=8))
        denp = es.enter_context(tc.tile_pool(name="den", bufs=2))
        smallp = es.enter_context(tc.tile_pool(name="small", bufs=2))
        yp = es.enter_context(tc.tile_pool(name="ysb", bufs=2))
        projp = es.enter_context(tc.tile_pool(name="pp", bufs=3, space="PSUM"))
        spsum = es.enter_context(tc.tile_pool(name="sp", bufs=3, space="PSUM"))
        opsum = es.enter_context(tc.tile_pool(name="op", bufs=2, space="PSUM"))
        if True:
            mask_t = []
            for m in range(4):
                mt = constp.tile([128, TB], BF16, tag=f"mask{m}")
                nc.sync.dma_start(mt[:], masks[m])
                mask_t.append(mt)
            ones_c = constp.tile([128, 1], BF16, tag="onesc")
            nc.sync.dma_start(ones_c[:], onescol[:])
            ones_r = constp.tile([1, 128], F32R, tag="onesr")
            nc.sync.dma_start(ones_r[:], onesrow[:])

            kT = [perp.tile([HD, T], BF16, tag=f"kT{h}", name=f"kT{h}") for h in range(NKV)]
            vT = [perp.tile([128, NKV * HD], BF16, tag=f"v{i}", name=f"v{i}") for i in range(T // 128)]

            def load_block(tb):
                tsl = slice(tb * TB, (tb + 1) * TB)
                xb = []
                for ct in range(NCT):
                    t_ = xp.tile([128, TB], BF16, tag="xb", name=f"xb{tb}_{ct}")
                    nc.sync.dma_start(t_[:], xT[ct * 128 : (ct + 1) * 128, tsl])
                    xb.append(t_)
                cosb = strp.tile([HD, TB], F32, tag="cosb", name=f"cosb{tb}")
                nc.sync.dma_start(cosb[:], cosdt[:, tsl])
                nsinb = strp.tile([HD, TB], F32, tag="nsinb", name=f"nsinb{tb}")
                nc.sync.dma_start(nsinb[:], nsindt[:, tsl])
                return xb, cosb, nsinb

            def proj_block(tb, xb, cosb, nsinb):
                tsl = slice(tb * TB, (tb + 1) * TB)
                # K projection (k^T layout [d, t]) + RoPE
                for kw in range(2):
                    kps = [projp.tile([128, TB], F32, tag="pp", name=f"kps{tb}_{kw}_{i}") for i in range(2)]
                    for ct in range(NCT):
                        wkt = wkp.tile([128, 256], BF16, tag="wk", name=f"wk{tb}_{kw}_{ct}")
                        nc.sync.dma_start(wkt[:], wkT[ct * 128 : (ct + 1) * 128, kw * 256 : (kw + 1) * 256])
                        for i in range(2):
                            nc.tensor.matmul(
                                kps[i][:],
                                wkt[:, i * 128 : (i + 1) * 128],
                                xb[ct][:],
                                start=(ct == 0),
                                stop=(ct == NCT - 1),
                            )
                    for i in range(2):
                        _rope(nc, tmpp, kT[kw * 2 + i][:, tsl], kps[i], cosb, nsinb)

                # V projection in [t, d] layout
                for vw in range(2):
                    vps = [projp.tile([128, NKV * HD], F32, tag="pp", name=f"vps{tb}_{vw}_{i}") for i in range(2)]
                    for ct in range(NCT):
                        wvt = wvp.tile([128, NKV * HD], BF16, tag="wv", name=f"wv{tb}_{vw}_{ct}")
                        nc.sync.dma_start(wvt[:], wvT[ct * 128 : (ct + 1) * 128, :])
                        for i in range(2):
                            nc.tensor.matmul(
                                vps[i][:],
                                xb[ct][:, (vw * 2 + i) * 128 : (vw * 2 + i + 1) * 128],
                                wvt[:],
                                start=(ct == 0),
                                stop=(ct == NCT - 1),
                            )
                    for i in range(2):
                        nc.vector.tensor_copy(vT[4 * tb + vw * 2 + i][:], vps[i][:])

                # Q projection (q^T layout) + RoPE, two waves of 4
                qts = []
                for wave in range(4):
                    qps = [projp.tile([128, TB], F32, tag="pp", name=f"qps{tb}_{wave}_{i}") for i in range(2)]
                    for ct in range(NCT):
                        wqt = wqp.tile([128, 256], BF16, tag="wq", name=f"wq{tb}_{wave}_{ct}")
                        nc.sync.dma_start(
                            wqt[:],
                            wqT[ct * 128 : (ct + 1) * 128, wave * 256 : (wave + 1) * 256],
                        )
                        for o in range(2):
                            nc.tensor.matmul(
                                qps[o][:],
                                wqt[:, o * 128 : (o + 1) * 128],
                                xb[ct][:],
                                start=(ct == 0),
                                stop=(ct == NCT - 1),
                            )
                    for o in range(2):
                        qt = qp.tile([HD, TB], BF16, tag="qt", name=f"qt{tb}_{wave}_{o}")
                        _rope(nc, tmpp, qt[:], qps[o], cosb, nsinb)
                        qts.append(qt)
                return qts

            def attention_block(tb, qts):
                ktmax = 4 * tb + 4
                outs = []
                tri = mask_t[0]  # [:, 0:128] is the lower-tri diagonal mask
                for h in range(NQH):
                    hv = h // 2
                    ops_ = opsum.tile([HD, TB], F32, tag="op", name=f"aop{tb}_{h}")
                    den = opsum.tile([1, TB], F32, tag="op", name=f"den{tb}_{h}")
                    for kt in range(ktmax):
                        sps = spsum.tile([128, TB], F32, tag="sp")
                        nc.tensor.matmul(
                            sps[:],
                            kT[hv][:, kt * 128 : (kt + 1) * 128],
                            qts[h][:],
                            start=True,
                            stop=True,
                        )
                        ex = expp.tile([128, TB], BF16, tag="exps")
                        nc.scalar.activation(ex[:], sps[:], EXP, scale=SCALE)
                        m = kt - 4 * tb
                        if m >= 0:
                            # causal: zero fully-masked q-subtiles (idle GpSimd)
                            # and apply the triangular mask on the diagonal one
                            if m > 0:
                                nc.gpsimd.memset(ex[:, 0 : 128 * m], 0.0)
                            nc.vector.tensor_mul(
                                ex[:, 128 * m : 128 * (m + 1)],
                                ex[:, 128 * m : 128 * (m + 1)],
                                tri[:, 0:128],
                            )
                        # denominator: accumulate ones.T @ ex on the PE in psum
                        nc.tensor.matmul(
                            den[:],
                            ones_c[:],
                            ex[:],
                            start=(kt == 0),
                            stop=(kt == ktmax - 1),
                        )
                        nc.tensor.matmul(
                            ops_[:],
                            vT[kt][:, hv * 128 : (hv + 1) * 128],
                            ex[:],
                            start=(kt == 0),
                            stop=(kt == ktmax - 1),
                        )
                    # single-op approx reciprocal (~18 bits, plenty), then
                    # partition-broadcast on the otherwise idle GpSimd engine
                    rec = smallp.tile([1, TB], F32, tag="rec")
                    nc.vector.reciprocal_approx_fast(rec[:], den[:])
                    bcs = smallp.tile([128, TB], F32, tag="bcs")
                    nc.gpsimd.partition_broadcast(bcs[:], rec[0:1, :])
                    ot = outp.tile([HD, TB], BF16, tag="ot")
                    nc.vector.tensor_mul(ot[:], ops_[:], bcs[:])
                    outs.append(ot)
                return outs

            def wo_block(tb, outs):
                tsl = slice(tb * TB, (tb + 1) * TB)
                for c2 in range(8):
                    yps = [projp.tile([128, TB], F32, tag="pp", name=f"yps{tb}_{c2}_{i}") for i in range(2)]
                    for jh in range(NQH):
                        wot = wop.tile([128, 256], BF16, tag="wo", name=f"wo{tb}_{c2}_{jh}")
                        nc.sync.dma_start(
                            wot[:],
                            woT[jh * 128 : (jh + 1) * 128, c2 * 256 : (c2 + 1) * 256],
                        )
                        for o in range(2):
                            nc.tensor.matmul(
                                yps[o][:],
                                wot[:, o * 128 : (o + 1) * 128],
                                outs[jh][:],
                                start=(jh == 0),
                                stop=(jh == NQH - 1),
                            )
                    for o in range(2):
                        ysb = yp.tile([128, TB], F32, tag="ysb")
                        nc.scalar.copy(ysb[:], yps[o][:])
                        og = c2 * 2 + o
                        nc.sync.dma_start(yT[og * 128 : (og + 1) * 128, tsl], ysb[:])

            # Software pipeline: attention/Wo of block tb-1 are emitted BEFORE
            # the projections of block tb, so the ACT-gated attention phase
            # always has dense projection matmuls to fill PE gaps (keeps the
            # HAM clock gate warm).
            prev_qts = None
            for tb in range(NTB):
                xb, cosb, nsinb = load_block(tb)
                if prev_qts is not None:
                    outs = attention_block(tb - 1, prev_qts)
                    wo_block(tb - 1, outs)
                prev_qts = proj_block(tb, xb, cosb, nsinb)
            outs = attention_block(NTB - 1, prev_qts)
            wo_block(NTB - 1, outs)

    nc.compile()
    return nc


def _host_consts():
    inv_freq = 1.0 / (10000.0 ** (np.arange(0, HD, 2, dtype=np.float32) / HD))
    t = np.arange(T, dtype=np.float32)
    freqs = np.outer(t, inv_freq)  # [T, HD/2]
    freqs = np.repeat(freqs, 2, axis=-1)  # [T, HD]
    cos = np.cos(freqs).astype(np.float32).T.copy()  # [HD, T]
    sin = np.sin(freqs).astype(np.float32).T.copy()
    # rotated-by-64 signed sin table: row d holds the multiplier that pairs
    # with x[(d+64)%128]; rows 64..127 carry -sin[0:64], rows 0..63 +sin[64:128]
    nsin = np.empty_like(sin)
    nsin[0:64, :] = sin[64:128, :]
    nsin[64:128, :] = -sin[0:64, :]

    masks = np.zeros((4, 128, TB), dtype=ml_dtypes.bfloat16)
    kp = np.arange(128)[:, None]
    qf = np.arange(TB)[None, :]
    for m in range(4):
        vis = (qf // 128 > m) | ((qf // 128 == m) & (kp <= qf % 128))
        masks[m] = vis.astype(ml_dtypes.bfloat16)

    return {
        "cosdt": np.ascontiguousarray(cos),
        "nsindt": np.ascontiguousarray(nsin),
        "masks": masks,
        "onescol": np.ones((128, 1), dtype=ml_dtypes.bfloat16),
        "onesrow": np.ones((1, 128), dtype=np.float32),
    }


_NC_CACHE = None


def _get_nc():
    global _NC_CACHE
    if _NC_CACHE is None:
        _NC_CACHE = build_nc()
    return _NC_CACHE


def kernel(x, Wq, Wk, Wv, Wo, _trace=False):
    x = np.asarray(x, dtype=np.float32)
    Wq = np.asarray(Wq, dtype=np.float32)
    Wk = np.asarray(Wk, dtype=np.float32)
    Wv = np.asarray(Wv, dtype=np.float32)
    Wo = np.asarray(Wo, dtype=np.float32)

    nc = _get_nc()
    consts = _host_consts()

    bf = ml_dtypes.bfloat16
    xTs = [np.ascontiguousarray(x[b].T.astype(bf)) for b in range(B)]
    wqTs = [np.ascontiguousarray(Wq[1024 * g : 1024 * (g + 1), :].T.astype(bf)) for g in range(2)]
    wkTs = [np.ascontiguousarray(Wk[512 * g : 512 * (g + 1), :].T.astype(bf)) for g in range(2)]
    wvTs = [np.ascontiguousarray(Wv[512 * g : 512 * (g + 1), :].T.astype(bf)) for g in range(2)]
    woTs = [np.ascontiguousarray(Wo[:, 1024 * g : 1024 * (g + 1)].T.astype(bf)) for g in range(2)]

    in_maps = []
    for c in range(8):
        b, g = c // 2, c % 2
        im = {
            "xT": xTs[b],
            "wqT": wqTs[g],
            "wkT": wkTs[g],
            "wvT": wvTs[g],
            "woT": woTs[g],
        }
        im.update(consts)
        in_maps.append(im)

    res = run_bass_kernel_spmd(nc, in_maps, core_ids=list(range(8)), trace=_trace)

    y = np.empty((B, T, C), dtype=np.float32)
    for b in range(B):
        y[b] = (res.results[2 * b]["yT"] + res.results[2 * b + 1]["yT"]).T
    if _trace:
        return y, res
    return y



# revision 2
# speedup vs baseline: 1.9449x; 1.9449x over previous
"""Causal GQA self-attention (B=4, T=2048, C=2048, 16 Q heads / 8 KV heads,
hd=128) as a Bass/Tile SPMD kernel on 8 Trainium2 NeuronCores.

Sharding: core c = (batch b = c//2, head-group g = c%2). Each core handles one
batch and 8 Q heads / 4 KV heads. Wq/Wk/Wv column-sharded on the head dim, Wo
row-sharded; the host sums the two partial Wo products per batch (2-way
all-reduce done on host during the gather).

All on-device tensors live in a transposed [feature, token] layout so every
matmul contraction sits on the partition dim with no on-device transposes:
  qT/kT = [d, t], v = [t, d], scores as S^T = [k, q], output as y^T = [o, t].
Bulk matmuls run in bf16 (fp32 PSUM accumulation; ~4e-3 end-to-end rel err).

v2: all four weight matrices are HOST-PRE-TILED into SBUF-resident layout and
DMAed ONCE at kernel start (4 large contiguous transfers) instead of being
re-streamed from HBM every token block through a 2-deep ring — the v1 profile
showed 245us of PE idle blocked on LDWEIGHTS waiting for weight DMAs, which
also kept re-tripping the HAM clock throttle (832us of the run at half PE
clock). x is likewise pre-tiled to one DMA per token block. The attention
inner loop is software-pipelined (scores+exp run PF tiles ahead of the
denominator/AV matmuls so the ScalarE exp latency is hidden), the softmax
denominator gets its own PSUM slot so a head's start doesn't wait on the
previous head's normalization chain, and scores/exp are column-restricted on
diagonal (causal-masked) k-tiles.
"""

import sys

import ml_dtypes
import numpy as np

sys.path.insert(0, "/opt/trn_rl_repo")

import concourse.bass as bass  # noqa: E402
import concourse.mybir as mybir  # noqa: E402
import concourse.tile as tile  # noqa: E402
from concourse import bacc  # noqa: E402
from concourse.bass_utils import run_bass_kernel_spmd  # noqa: E402

# Problem shape (hardcoded per contest contract).
B = 4
T = 2048
C = 2048
HD = 128
N_HEAD = 16
N_KV_HEAD = 8
NQH = N_HEAD // 2  # q heads per core (group)
NKV = N_KV_HEAD // 2  # kv heads per core
TB = 512  # token block
NTB = T // TB
NCT = C // 128  # contraction tiles for the projections
SCALE = 1.0 / float(np.sqrt(HD))
PF = 2  # scores/exp prefetch depth in the attention pipeline

F32 = mybir.dt.float32
BF16 = mybir.dt.bfloat16
MULT = mybir.AluOpType.mult
ADD = mybir.AluOpType.add
EXP = mybir.ActivationFunctionType.Exp


def _rope(nc, tmpp, dst, src_psum, cosb, nsinb):
    """dst = src*cos + rot_half(src)*sin, src in [d, t] layout (d partitions).

    rot_half(x)[d] = -x[d+64] for d<64, +x[d-64] for d>=64; the sign lives in
    nsinb so both halves are plain multiplies. nsinb is the sin table rotated
    by 64 partitions (nsinb[64+i] = -sin[i], nsinb[i] = sin[64+i]) so each
    tensor_tensor has equal base partitions on its two SBUF inputs (HW rule).
    """
    t0 = tmpp.tile([HD, TB], F32, tag="t0")
    nc.scalar.copy(t0[:], src_psum[:])
    nc.vector.tensor_mul(dst, t0[:], cosb[:])
    t2 = tmpp.tile([HD, TB], F32, tag="t2")
    nc.vector.tensor_mul(t2[0:64, :], t0[64:128, :], nsinb[64:128, :])
    nc.vector.tensor_mul(t2[64:128, :], t0[0:64, :], nsinb[0:64, :])
    nc.vector.scalar_tensor_tensor(dst, t2[:], 1.0, dst, op0=MULT, op1=ADD)


def build_nc():
    nc = bacc.Bacc("TRN2", target_bir_lowering=False, debug=False, num_devices=8)

    # host-pre-tiled inputs (see kernel() for the exact layouts)
    xt = nc.dram_tensor("xt", [NTB, 128, NCT * TB], BF16, kind="ExternalInput")
    wq = nc.dram_tensor("wq", [128, NCT * 1024], BF16, kind="ExternalInput")
    wk = nc.dram_tensor("wk", [128, NCT * 512], BF16, kind="ExternalInput")
    wv = nc.dram_tensor("wv", [128, NCT * 512], BF16, kind="ExternalInput")
    wo = nc.dram_tensor("wo", [128, NQH * 2048], BF16, kind="ExternalInput")
    cosdt = nc.dram_tensor("cosdt", [HD, T], F32, kind="ExternalInput")
    nsindt = nc.dram_tensor("nsindt", [HD, T], F32, kind="ExternalInput")
    trid = nc.dram_tensor("trid", [128, 128], BF16, kind="ExternalInput")
    onescol = nc.dram_tensor("onescol", [128, 1], BF16, kind="ExternalInput")
    yT = nc.dram_tensor("yT", [C, T], F32, kind="ExternalOutput")

    from contextlib import ExitStack

    with ExitStack() as es:
        tc = es.enter_context(tile.TileContext(nc))
        es.enter_context(nc.allow_low_precision("fp32r attention"))
        constp = es.enter_context(tc.tile_pool(name="const", bufs=1))
        strp = es.enter_context(tc.tile_pool(name="stream", bufs=2))
        perp = es.enter_context(tc.tile_pool(name="persist", bufs=1))
        xp = es.enter_context(tc.tile_pool(name="xp", bufs=1))
        qp = es.enter_context(tc.tile_pool(name="qt", bufs=12))
        outp = es.enter_context(tc.tile_pool(name="ot", bufs=8))
        tmpp = es.enter_context(tc.tile_pool(name="tmp", bufs=2))
        expp = es.enter_context(tc.tile_pool(name="exps", bufs=6))
        smallp = es.enter_context(tc.tile_pool(name="small", bufs=2))
        yp = es.enter_context(tc.tile_pool(name="ysb", bufs=2))
        projp = es.enter_context(tc.tile_pool(name="pp", bufs=3, space="PSUM"))
        spsum = es.enter_context(tc.tile_pool(name="sp", bufs=PF, space="PSUM"))
        opsum = es.enter_context(tc.tile_pool(name="op", bufs=2, space="PSUM"))
        denp = es.enter_context(tc.tile_pool(name="dp", bufs=1, space="PSUM"))
        if True:
            tri = constp.tile([128, 128], BF16, tag="tri")
            nc.sync.dma_start(tri[:], trid[:])
            ones_c = constp.tile([128, 1], BF16, tag="onesc")
            nc.sync.dma_start(ones_c[:], onescol[:])
            # resident weights: one contiguous DMA each, issued in first-use
            # order (K needs wk+x first; wq must land by ~28us, wo by ~75us)
            wk_t = constp.tile([128, NCT * 512], BF16, tag="wk")
            nc.sync.dma_start(wk_t[:], wk[:])
            wv_t = constp.tile([128, NCT * 512], BF16, tag="wv")
            nc.sync.dma_start(wv_t[:], wv[:])
            wq_t = constp.tile([128, NCT * 1024], BF16, tag="wq")
            nc.sync.dma_start(wq_t[:], wq[:])
            wo_t = constp.tile([128, NQH * 2048], BF16, tag="wo")
            nc.sync.dma_start(wo_t[:], wo[:])

            kT = [perp.tile([HD, T], BF16, tag=f"kT{h}", name=f"kT{h}") for h in range(NKV)]
            vT = [perp.tile([128, NKV * HD], BF16, tag=f"v{i}", name=f"v{i}") for i in range(T // 128)]

            def load_block(tb):
                tsl = slice(tb * TB, (tb + 1) * TB)
                xblk = xp.tile([128, NCT * TB], BF16, tag="xblk", name=f"xblk{tb}")
                nc.sync.dma_start(xblk[:], xt[tb])
                cosb = strp.tile([HD, TB], F32, tag="cosb", name=f"cosb{tb}")
                nc.sync.dma_start(cosb[:], cosdt[:, tsl])
                nsinb = strp.tile([HD, TB], F32, tag="nsinb", name=f"nsinb{tb}")
                nc.sync.dma_start(nsinb[:], nsindt[:, tsl])
                return xblk, cosb, nsinb

            def proj_block(tb, xblk, cosb, nsinb):
                tsl = slice(tb * TB, (tb + 1) * TB)
                # K projection (k^T layout [d, t]) + RoPE
                for kw in range(2):
                    kps = [projp.tile([128, TB], F32, tag="pp", name=f"kps{tb}_{kw}_{i}") for i in range(2)]
                    for ct in range(NCT):
                        wcol = ct * 512 + kw * 256
                        for i in range(2):
                            nc.tensor.matmul(
                                kps[i][:],
                                wk_t[:, wcol + i * 128 : wcol + (i + 1) * 128],
                                xblk[:, ct * TB : (ct + 1) * TB],
                                start=(ct == 0),
                                stop=(ct == NCT - 1),
                            )
                    for i in range(2):
                        _rope(nc, tmpp, kT[kw * 2 + i][:, tsl], kps[i], cosb, nsinb)

                # V projection in [t, d] layout
                for vw in range(2):
                    vps = [projp.tile([128, NKV * HD], F32, tag="pp", name=f"vps{tb}_{vw}_{i}") for i in range(2)]
                    for ct in range(NCT):
                        for i in range(2):
                            tcol = ct * TB + (vw * 2 + i) * 128
                            nc.tensor.matmul(
                                vps[i][:],
                                xblk[:, tcol : tcol + 128],
                                wv_t[:, ct * 512 : (ct + 1) * 512],
                                start=(ct == 0),
                                stop=(ct == NCT - 1),
                            )
                    for i in range(2):
                        nc.vector.tensor_copy(vT[4 * tb + vw * 2 + i][:], vps[i][:])

                # Q projection (q^T layout) + RoPE, four waves of 2
                qts = []
                for wave in range(4):
                    qps = [projp.tile([128, TB], F32, tag="pp", name=f"qps{tb}_{wave}_{i}") for i in range(2)]
                    for ct in range(NCT):
                        wcol = ct * 1024 + wave * 256
                        for o in range(2):
                            nc.tensor.matmul(
                                qps[o][:],
                                wq_t[:, wcol + o * 128 : wcol + (o + 1) * 128],
                                xblk[:, ct * TB : (ct + 1) * TB],
                                start=(ct == 0),
                                stop=(ct == NCT - 1),
                            )
                    for o in range(2):
                        qt = qp.tile([HD, TB], BF16, tag="qt", name=f"qt{tb}_{wave}_{o}")
                        _rope(nc, tmpp, qt[:], qps[o], cosb, nsinb)
                        qts.append(qt)
                return qts

            def attention_block(tb, qts):
                ktmax = 4 * tb + 4
                outs = []
                for h in range(NQH):
                    hv = h // 2
                    ops_ = opsum.tile([HD, TB], F32, tag="op", name=f"aop{tb}_{h}")
                    den = denp.tile([1, TB], F32, tag="dp", name=f"den{tb}_{h}")

                    def emit_s(kt, h=h, hv=hv):
                        # scores + exp for one k-tile; on diagonal tiles only
                        # the live q columns are computed, the rest memset 0
                        m = kt - 4 * tb
                        q0 = 128 * m if m > 0 else 0
                        sps = spsum.tile([128, TB], F32, tag="sp")
                        nc.tensor.matmul(
                            sps[:, q0:TB],
                            kT[hv][:, kt * 128 : (kt + 1) * 128],
                            qts[h][:, q0:TB],
                            start=True,
                            stop=True,
                        )
                        ex = expp.tile([128, TB], BF16, tag="exps")
                        nc.scalar.activation(ex[:, q0:TB], sps[:, q0:TB], EXP, scale=SCALE)
                        if m >= 0:
                            if m > 0:
                                nc.gpsimd.memset(ex[:, 0:q0], 0.0)
                            nc.vector.tensor_mul(
                                ex[:, q0 : q0 + 128],
                                ex[:, q0 : q0 + 128],
                                tri[:],
                            )
                        return ex

                    exq = [emit_s(kt) for kt in range(min(PF, ktmax))]
                    for kt in range(ktmax):
                        if kt + PF < ktmax:
                            exq.append(emit_s(kt + PF))
                        ex = exq[kt]
                        # denominator: accumulate ones.T @ ex on the PE
                        nc.tensor.matmul(
                            den[:],
                            ones_c[:],
                            ex[:],
                            start=(kt == 0),
                            stop=(kt == ktmax - 1),
                        )
                        nc.tensor.matmul(
                            ops_[:],
                            vT[kt][:, hv * 128 : (hv + 1) * 128],
                            ex[:],
                            start=(kt == 0),
                            stop=(kt == ktmax - 1),
                        )
                    # single-op approx reciprocal (~18 bits, plenty), then
                    # partition-broadcast on the otherwise idle GpSimd engine
                    rec = smallp.tile([1, TB], F32, tag="rec")
                    nc.vector.reciprocal_approx_fast(rec[:], den[:])
                    bcs = smallp.tile([128, TB], F32, tag="bcs")
                    nc.gpsimd.partition_broadcast(bcs[:], rec[0:1, :])
                    ot = outp.tile([HD, TB], BF16, tag="ot")
                    nc.vector.tensor_mul(ot[:], ops_[:], bcs[:])
                    outs.append(ot)
                return outs

            def wo_block(tb, outs):
                tsl = slice(tb * TB, (tb + 1) * TB)
                for c2 in range(8):
                    yps = [projp.tile([128, TB], F32, tag="pp", name=f"yps{tb}_{c2}_{i}") for i in range(2)]
                    for jh in range(NQH):
                        wcol = jh * 2048 + c2 * 256
                        for o in range(2):
                            nc.tensor.matmul(
                                yps[o][:],
                                wo_t[:, wcol + o * 128 : wcol + (o + 1) * 128],
                                outs[jh][:],
                                start=(jh == 0),
                                stop=(jh == NQH - 1),
                            )
                    for o in range(2):
                        ysb = yp.tile([128, TB], F32, tag="ysb")
                        nc.scalar.copy(ysb[:], yps[o][:])
                        og = c2 * 2 + o
                        nc.sync.dma_start(yT[og * 128 : (og + 1) * 128, tsl], ysb[:])

            # Software pipeline: attention/Wo of block tb-1 are emitted BEFORE
            # the projections of block tb, so DMAs for block tb issue early and
            # the PE stream stays dense across phase boundaries.
            prev_qts = None
            for tb in range(NTB):
                xblk, cosb, nsinb = load_block(tb)
                if prev_qts is not None:
                    outs = attention_block(tb - 1, prev_qts)
                    wo_block(tb - 1, outs)
                prev_qts = proj_block(tb, xblk, cosb, nsinb)
            outs = attention_block(NTB - 1, prev_qts)
            wo_block(NTB - 1, outs)

    nc.compile()
    return nc


def _host_consts():
    inv_freq = 1.0 / (10000.0 ** (np.arange(0, HD, 2, dtype=np.float32) / HD))
    t = np.arange(T, dtype=np.float32)
    freqs = np.outer(t, inv_freq)  # [T, HD/2]
    freqs = np.repeat(freqs, 2, axis=-1)  # [T, HD]
    cos = np.cos(freqs).astype(np.float32).T.copy()  # [HD, T]
    sin = np.sin(freqs).astype(np.float32).T.copy()
    # rotated-by-64 signed sin table: row d holds the multiplier that pairs
    # with x[(d+64)%128]; rows 64..127 carry -sin[0:64], rows 0..63 +sin[64:128]
    nsin = np.empty_like(sin)
    nsin[0:64, :] = sin[64:128, :]
    nsin[64:128, :] = -sin[0:64, :]

    kp = np.arange(128)[:, None]
    qf = np.arange(128)[None, :]
    tri = (kp <= qf).astype(ml_dtypes.bfloat16)

    return {
        "cosdt": np.ascontiguousarray(cos),
        "nsindt": np.ascontiguousarray(nsin),
        "trid": tri,
        "onescol": np.ones((128, 1), dtype=ml_dtypes.bfloat16),
    }


_NC_CACHE = None


def _get_nc():
    global _NC_CACHE
    if _NC_CACHE is None:
        _NC_CACHE = build_nc()
    return _NC_CACHE


def kernel(x, Wq, Wk, Wv, Wo, _trace=False):
    x = np.asarray(x, dtype=np.float32)
    Wq = np.asarray(Wq, dtype=np.float32)
    Wk = np.asarray(Wk, dtype=np.float32)
    Wv = np.asarray(Wv, dtype=np.float32)
    Wo = np.asarray(Wo, dtype=np.float32)

    nc = _get_nc()
    consts = _host_consts()

    bf = ml_dtypes.bfloat16
    # x pre-tiled per batch: xt[tb][p][ct*TB + t] = x[b, tb*TB + t, ct*128 + p]
    xts = [
        np.ascontiguousarray(
            x[b].astype(bf).reshape(NTB, TB, NCT, 128).transpose(0, 3, 2, 1).reshape(NTB, 128, NCT * TB)
        )
        for b in range(B)
    ]

    def _tile_w(A, w):  # A: [dout, C] -> [128, NCT*w], w = dout per group
        # out[p, ct*w + j] = A[j, ct*128 + p]
        return np.ascontiguousarray(A.T.reshape(NCT, 128, w).transpose(1, 0, 2).reshape(128, NCT * w)).astype(bf)

    wqs = [_tile_w(Wq[1024 * g : 1024 * (g + 1), :], 1024) for g in range(2)]
    wks = [_tile_w(Wk[512 * g : 512 * (g + 1), :], 512) for g in range(2)]
    wvs = [_tile_w(Wv[512 * g : 512 * (g + 1), :], 512) for g in range(2)]
    # wo[p, jh*2048 + j] = Wo[j, 1024g + jh*128 + p]
    wos = [
        np.ascontiguousarray(
            Wo[:, 1024 * g : 1024 * (g + 1)].T.reshape(NQH, 128, 2048).transpose(1, 0, 2).reshape(128, NQH * 2048)
        ).astype(bf)
        for g in range(2)
    ]

    in_maps = []
    for c in range(8):
        b, g = c // 2, c % 2
        im = {
            "xt": xts[b],
            "wq": wqs[g],
            "wk": wks[g],
            "wv": wvs[g],
            "wo": wos[g],
        }
        im.update(consts)
        in_maps.append(im)

    res = run_bass_kernel_spmd(nc, in_maps, core_ids=list(range(8)), trace=_trace)

    y = np.empty((B, T, C), dtype=np.float32)
    for b in range(B):
        y[b] = (res.results[2 * b]["yT"] + res.results[2 * b + 1]["yT"]).T
    if _trace:
        return y, res
    return y


# revision 4
# speedup vs baseline: 2.3993x; 1.2337x over previous
"""Causal GQA self-attention (B=4, T=2048, C=2048, 16 Q heads / 8 KV heads,
hd=128) as a Bass/Tile SPMD kernel on 8 Trainium2 NeuronCores.

Sharding: core c = (batch b = c//2, head-group g = c%2). Each core handles one
batch and 8 Q heads / 4 KV heads. Wq/Wk/Wv column-sharded on the head dim, Wo
row-sharded; the host sums the two partial Wo products per batch (2-way
all-reduce done on host during the gather).

All on-device tensors live in a transposed [feature, token] layout so every
matmul contraction sits on the partition dim with no on-device transposes:
  qT/kT = [d, t], v = [t, d], scores as S^T = [k, q], output as y^T = [o, t].
Bulk matmuls run in bf16 (fp32 PSUM accumulation; ~4e-3 end-to-end rel err).

v3: Wq/Wk/Wv are host-pre-tiled and DMAed once into resident SBUF (Wo streams
per block in c2-major tiles); x is pre-tiled to 4 chunked DMAs per block. The
softmax denominator is accumulated on the DVE (acc += ex per k-tile, fp32)
with ONE ones-matmul per head instead of one per k-tile, removing 320 PE
passes. Scores/exp/AV/accumulate are all column-restricted on diagonal
(causal) k-tiles, so the GpSimd memsets are gone entirely. Projection and Wo
matmul work is chopped into small chunks by Python generators and pumped into
the exp-paced attention emission stream (attention(t) interleaves proj(t+1)
and wo(t-1) chunks), so the in-order PE queue always has dense work while the
ScalarE exp stream drains. Per-head normalization (reciprocal + GpSimd
partition-broadcast + multiply) is deferred one head so its latency hides.
"""

import sys

import ml_dtypes
import numpy as np

sys.path.insert(0, "/opt/trn_rl_repo")

import concourse.bass as bass  # noqa: E402
import concourse.mybir as mybir  # noqa: E402
import concourse.tile as tile  # noqa: E402
from concourse import bacc  # noqa: E402
from concourse.bass_utils import run_bass_kernel_spmd  # noqa: E402

# Problem shape (hardcoded per contest contract).
B = 4
T = 2048
C = 2048
HD = 128
N_HEAD = 16
N_KV_HEAD = 8
NQH = N_HEAD // 2  # q heads per core (group)
NKV = N_KV_HEAD // 2  # kv heads per core
TB = 512  # token block
NTB = T // TB
NCT = C // 128  # contraction tiles for the projections
SCALE = 1.0 / float(np.sqrt(HD))
PF = 2  # scores/exp prefetch depth in the attention pipeline
WO_START_ITER = 10  # delay wo pops until its streamed weights have landed

F32 = mybir.dt.float32
BF16 = mybir.dt.bfloat16
MULT = mybir.AluOpType.mult
ADD = mybir.AluOpType.add
EXP = mybir.ActivationFunctionType.Exp


def _rope(nc, tmpp, dst, src_psum, cosb, nsinb):
    """dst = src*cos + rot_half(src)*sin, src in [d, t] layout (d partitions).

    rot_half(x)[d] = -x[d+64] for d<64, +x[d-64] for d>=64; the sign lives in
    nsinb so both halves are plain multiplies. nsinb is the sin table rotated
    by 64 partitions (nsinb[64+i] = -sin[i], nsinb[i] = sin[64+i]) so each
    tensor_tensor has equal base partitions on its two SBUF inputs (HW rule).
    """
    t0 = tmpp.tile([HD, TB], F32, tag="t0")
    nc.scalar.copy(t0[:], src_psum[:])
    nc.vector.tensor_mul(dst, t0[:], cosb[:])
    t2 = tmpp.tile([HD, TB], F32, tag="t2")
    nc.vector.tensor_mul(t2[0:64, :], t0[64:128, :], nsinb[64:128, :])
    nc.vector.tensor_mul(t2[64:128, :], t0[0:64, :], nsinb[0:64, :])
    nc.vector.scalar_tensor_tensor(dst, t2[:], 1.0, dst, op0=MULT, op1=ADD)


def build_nc():
    nc = bacc.Bacc("TRN2", target_bir_lowering=False, debug=False, num_devices=8)

    # host-pre-tiled inputs (see kernel() for the exact layouts)
    xt = nc.dram_tensor("xt", [NTB, 128, NCT * TB], BF16, kind="ExternalInput")
    wq = nc.dram_tensor("wq", [128, NCT * 1024], BF16, kind="ExternalInput")
    wk = nc.dram_tensor("wk", [128, NCT * 512], BF16, kind="ExternalInput")
    wv = nc.dram_tensor("wv", [128, NCT * 512], BF16, kind="ExternalInput")
    wo = nc.dram_tensor("wo", [128, 8 * 2048], BF16, kind="ExternalInput")
    cosdt = nc.dram_tensor("cosdt", [HD, T], F32, kind="ExternalInput")
    nsindt = nc.dram_tensor("nsindt", [HD, T], F32, kind="ExternalInput")
    trid = nc.dram_tensor("trid", [128, 128], BF16, kind="ExternalInput")
    onescol = nc.dram_tensor("onescol", [128, 1], BF16, kind="ExternalInput")
    yT = nc.dram_tensor("yT", [C, T], F32, kind="ExternalOutput")

    from contextlib import ExitStack

    with ExitStack() as es:
        tc = es.enter_context(tile.TileContext(nc))
        es.enter_context(nc.allow_low_precision("fp32r attention"))
        constp = es.enter_context(tc.tile_pool(name="const", bufs=1))
        strp = es.enter_context(tc.tile_pool(name="stream", bufs=2))
        perp = es.enter_context(tc.tile_pool(name="persist", bufs=1))
        xp = es.enter_context(tc.tile_pool(name="xp", bufs=1))
        wop = es.enter_context(tc.tile_pool(name="wop", bufs=3))
        qp = es.enter_context(tc.tile_pool(name="qt", bufs=10))
        outp = es.enter_context(tc.tile_pool(name="ot", bufs=8))
        tmpp = es.enter_context(tc.tile_pool(name="tmp", bufs=2))
        expp = es.enter_context(tc.tile_pool(name="exps", bufs=6))
        accp = es.enter_context(tc.tile_pool(name="acc", bufs=2))
        smallp = es.enter_context(tc.tile_pool(name="small", bufs=2))
        yp = es.enter_context(tc.tile_pool(name="ysb", bufs=2))
        projp = es.enter_context(tc.tile_pool(name="pp", bufs=3, space="PSUM"))
        spsum = es.enter_context(tc.tile_pool(name="sp", bufs=PF, space="PSUM"))
        opsum = es.enter_context(tc.tile_pool(name="op", bufs=2, space="PSUM"))
        denp = es.enter_context(tc.tile_pool(name="dp", bufs=1, space="PSUM"))
        if True:
            tri = constp.tile([128, 128], BF16, tag="tri")
            nc.sync.dma_start(tri[:], trid[:])
            ones_c = constp.tile([128, 1], BF16, tag="onesc")
            nc.sync.dma_start(ones_c[:], onescol[:])
            # resident weights; wk chunk-interleaved with x chunks of block 0
            # (emitted in load_block below) so the first K matmuls start ~3us
            # in instead of waiting for the whole preload
            wk_t = constp.tile([128, NCT * 512], BF16, tag="wk")
            wv_t = constp.tile([128, NCT * 512], BF16, tag="wv")
            wq_t = constp.tile([128, NCT * 1024], BF16, tag="wq")

            kT = [perp.tile([HD, T], BF16, tag=f"kT{h}", name=f"kT{h}") for h in range(NKV)]
            vT = [perp.tile([128, NKV * HD], BF16, tag=f"v{i}", name=f"v{i}") for i in range(T // 128)]

            def load_block(tb):
                tsl = slice(tb * TB, (tb + 1) * TB)
                xblk = xp.tile([128, NCT * TB], BF16, tag="xblk", name=f"xblk{tb}")
                qtr = (NCT * TB) // 4
                for ch in range(4):
                    nc.sync.dma_start(xblk[:, ch * qtr : (ch + 1) * qtr], xt[tb][:, ch * qtr : (ch + 1) * qtr])
                    if tb == 0:
                        wqt = (NCT * 512) // 4
                        nc.sync.dma_start(wk_t[:, ch * wqt : (ch + 1) * wqt], wk[:, ch * wqt : (ch + 1) * wqt])
                cosb = strp.tile([HD, TB], F32, tag="cosb", name=f"cosb{tb}")
                nc.sync.dma_start(cosb[:], cosdt[:, tsl])
                nsinb = strp.tile([HD, TB], F32, tag="nsinb", name=f"nsinb{tb}")
                nc.sync.dma_start(nsinb[:], nsindt[:, tsl])
                if tb == 0:
                    nc.sync.dma_start(wv_t[:], wv[:])
                    nc.sync.dma_start(wq_t[:], wq[:])
                return xblk, cosb, nsinb

            def proj_gen(tb, xblk, cosb, nsinb, qts_out):
                """Generator emitting the projections of block tb in ~8-matmul
                chunks; fills qts_out with the 8 roped q tiles."""
                tsl = slice(tb * TB, (tb + 1) * TB)
                # K projection (k^T layout [d, t]) + RoPE
                for kw in range(2):
                    kps = [projp.tile([128, TB], F32, tag="pp", name=f"kps{tb}_{kw}_{i}") for i in range(2)]
                    for ct in range(NCT):
                        wcol = ct * 512 + kw * 256
                        for i in range(2):
                            nc.tensor.matmul(
                                kps[i][:],
                                wk_t[:, wcol + i * 128 : wcol + (i + 1) * 128],
                                xblk[:, ct * TB : (ct + 1) * TB],
                                start=(ct == 0),
                                stop=(ct == NCT - 1),
                            )
                        if ct % 4 == 3 and ct < NCT - 1:
                            yield
                    for i in range(2):
                        _rope(nc, tmpp, kT[kw * 2 + i][:, tsl], kps[i], cosb, nsinb)
                    yield

                # V projection in [t, d] layout; psum evacuated on ScalarE so
                # the DVE rope stream can't back up the projp psum ring
                for vw in range(2):
                    vps = [projp.tile([128, NKV * HD], F32, tag="pp", name=f"vps{tb}_{vw}_{i}") for i in range(2)]
                    for ct in range(NCT):
                        for i in range(2):
                            tcol = ct * TB + (vw * 2 + i) * 128
                            nc.tensor.matmul(
                                vps[i][:],
                                xblk[:, tcol : tcol + 128],
                                wv_t[:, ct * 512 : (ct + 1) * 512],
                                start=(ct == 0),
                                stop=(ct == NCT - 1),
                            )
                        if ct % 4 == 3 and ct < NCT - 1:
                            yield
                    for i in range(2):
                        nc.scalar.copy(vT[4 * tb + vw * 2 + i][:], vps[i][:])
                    yield

                # Q projection (q^T layout) + RoPE, four waves of 2
                for wave in range(4):
                    qps = [projp.tile([128, TB], F32, tag="pp", name=f"qps{tb}_{wave}_{i}") for i in range(2)]
                    for ct in range(NCT):
                        wcol = ct * 1024 + wave * 256
                        for o in range(2):
                            nc.tensor.matmul(
                                qps[o][:],
                                wq_t[:, wcol + o * 128 : wcol + (o + 1) * 128],
                                xblk[:, ct * TB : (ct + 1) * TB],
                                start=(ct == 0),
                                stop=(ct == NCT - 1),
                            )
                        if ct % 4 == 3 and ct < NCT - 1:
                            yield
                    for o in range(2):
                        qt = qp.tile([HD, TB], BF16, tag="qt", name=f"qt{tb}_{wave}_{o}")
                        _rope(nc, tmpp, qt[:], qps[o], cosb, nsinb)
                        qts_out.append(qt)
                    yield

            def wo_gen(tb, outs):
                """Generator emitting wo @ outs(tb) in half-c2 chunks. Weights
                stream per block as c2-major [128, 2048] tiles (ring 3)."""
                tsl = slice(tb * TB, (tb + 1) * TB)
                tiles = {}

                def load(c2):
                    t_ = wop.tile([128, 2048], BF16, tag="wo", name=f"wo{tb}_{c2}")
                    nc.sync.dma_start(t_[:], wo[:, c2 * 2048 : (c2 + 1) * 2048])
                    tiles[c2] = t_

                for c2 in range(3):
                    load(c2)
                for c2 in range(8):
                    yps = [projp.tile([128, TB], F32, tag="pp", name=f"yps{tb}_{c2}_{i}") for i in range(2)]
                    wt = tiles.pop(c2)
                    for jh in range(NQH):
                        wcol = jh * 256
                        for o in range(2):
                            nc.tensor.matmul(
                                yps[o][:],
                                wt[:, wcol + o * 128 : wcol + (o + 1) * 128],
                                outs[jh][:],
                                start=(jh == 0),
                                stop=(jh == NQH - 1),
                            )
                        if jh == 3:
                            if c2 + 3 < 8:
                                load(c2 + 3)
                            yield
                    for o in range(2):
                        ysb = yp.tile([128, TB], F32, tag="ysb")
                        nc.scalar.copy(ysb[:], yps[o][:])
                        og = c2 * 2 + o
                        nc.sync.dma_start(yT[og * 128 : (og + 1) * 128, tsl], ysb[:])
                    yield

            def attention_block(tb, qts, wgen, pgen, n_wo, n_proj):
                """Attention of block tb, pumping chunks from wo(tb-1) and
                proj(tb+1) generators into the PE stream between iterations."""
                ktmax = 4 * tb + 4
                iters_total = NQH * ktmax
                total_chunks = n_wo + n_proj
                rate = total_chunks / iters_total
                state = {"q": 0.0, "it": 0, "w": wgen, "p": pgen, "wi": 0}

                def pop_one():
                    # prefer wo once past WO_START_ITER (2 proj : 1 wo), else proj
                    order = []
                    if state["it"] >= WO_START_ITER and state["w"] is not None:
                        if state["wi"] % 3 == 0:
                            order = ["w", "p"]
                        else:
                            order = ["p", "w"]
                        state["wi"] += 1
                    else:
                        order = ["p", "w"] if state["it"] >= WO_START_ITER else ["p"]
                    for k in order:
                        g = state[k]
                        if g is None:
                            continue
                        try:
                            next(g)
                            return True
                        except StopIteration:
                            state[k] = None
                    return False

                def pump():
                    state["it"] += 1
                    state["q"] += rate
                    while state["q"] >= 1.0:
                        state["q"] -= 1.0
                        if not pop_one():
                            state["q"] = 0.0
                            break

                outs = [None] * NQH
                pending = None  # (h, ops_, bcs) of previous head
                for h in range(NQH):
                    hv = h // 2
                    ops_ = opsum.tile([HD, TB], F32, tag="op", name=f"aop{tb}_{h}")
                    acc = accp.tile([128, TB], BF16, tag="acc", name=f"acc{tb}_{h}")

                    def emit_s(kt, h=h, hv=hv):
                        # scores + exp for one k-tile; on diagonal tiles only
                        # the live q columns [q0:TB] are computed/consumed
                        m = kt - 4 * tb
                        q0 = 128 * m if m > 0 else 0
                        sps = spsum.tile([128, TB], F32, tag="sp")
                        nc.tensor.matmul(
                            sps[:, q0:TB],
                            kT[hv][:, kt * 128 : (kt + 1) * 128],
                            qts[h][:, q0:TB],
                            start=True,
                            stop=True,
                        )
                        ex = expp.tile([128, TB], BF16, tag="exps")
                        nc.scalar.activation(ex[:, q0:TB], sps[:, q0:TB], EXP, scale=SCALE)
                        if m >= 0:
                            nc.vector.tensor_mul(
                                ex[:, q0 : q0 + 128],
                                ex[:, q0 : q0 + 128],
                                tri[:],
                            )
                        return ex, q0

                    exq = [emit_s(kt) for kt in range(min(PF, ktmax))]
                    for kt in range(ktmax):
                        if kt + PF < ktmax:
                            exq.append(emit_s(kt + PF))
                        ex, q0 = exq[kt]
                        # denominator partial sums accumulate on the DVE
                        if kt == 0:
                            nc.vector.tensor_copy(acc[:], ex[:])
                        else:
                            nc.vector.tensor_add(acc[:, q0:TB], acc[:, q0:TB], ex[:, q0:TB])
                        nc.tensor.matmul(
                            ops_[:, q0:TB],
                            vT[kt][:, hv * 128 : (hv + 1) * 128],
                            ex[:, q0:TB],
                            start=(kt == 0),
                            stop=(kt == ktmax - 1),
                        )
                        pump()
                    # one ones-matmul per head closes the denominator, then
                    # reciprocal + partition-broadcast; the ops_*rec multiply
                    # is deferred one head so the GpSimd broadcast latency hides
                    den = denp.tile([1, TB], F32, tag="dp", name=f"den{tb}_{h}")
                    nc.tensor.matmul(den[:], ones_c[:], acc[:], start=True, stop=True)
                    rec = smallp.tile([1, TB], F32, tag="rec")
                    nc.vector.reciprocal_approx_fast(rec[:], den[:])
                    bcs = smallp.tile([128, TB], F32, tag="bcs")
                    nc.gpsimd.partition_broadcast(bcs[:], rec[0:1, :])
                    if pending is not None:
                        ph, pops_, pbcs = pending
                        ot = outp.tile([HD, TB], BF16, tag="ot")
                        nc.vector.tensor_mul(ot[:], pops_[:], pbcs[:])
                        outs[ph] = ot
                    pending = (h, ops_, bcs)
                ph, pops_, pbcs = pending
                ot = outp.tile([HD, TB], BF16, tag="ot")
                nc.vector.tensor_mul(ot[:], pops_[:], pbcs[:])
                outs[ph] = ot
                # drain any remaining interleave chunks
                while pop_one():
                    pass
                return outs

            # Pipeline: P0 | A0+P1 | A1+P2+W0 | A2+P3+W1 | A3+W2 | W3
            qts_all = {}
            xblk, cosb, nsinb = load_block(0)
            qts_all[0] = []
            for _ in proj_gen(0, xblk, cosb, nsinb, qts_all[0]):
                pass
            outs_prev = None
            for t in range(NTB):
                pgen = None
                n_proj = 0
                if t + 1 < NTB:
                    xblk, cosb, nsinb = load_block(t + 1)
                    qts_all[t + 1] = []
                    pgen = proj_gen(t + 1, xblk, cosb, nsinb, qts_all[t + 1])
                    n_proj = 32
                wgen = None
                n_wo = 0
                if outs_prev is not None:
                    wgen = wo_gen(t - 1, outs_prev)
                    n_wo = 16
                outs_prev = attention_block(t, qts_all[t], wgen, pgen, n_wo, n_proj)
            for _ in wo_gen(NTB - 1, outs_prev):
                pass

    nc.compile()
    return nc


def _host_consts():
    inv_freq = 1.0 / (10000.0 ** (np.arange(0, HD, 2, dtype=np.float32) / HD))
    t = np.arange(T, dtype=np.float32)
    freqs = np.outer(t, inv_freq)  # [T, HD/2]
    freqs = np.repeat(freqs, 2, axis=-1)  # [T, HD]
    cos = np.cos(freqs).astype(np.float32).T.copy()  # [HD, T]
    sin = np.sin(freqs).astype(np.float32).T.copy()
    # rotated-by-64 signed sin table: row d holds the multiplier that pairs
    # with x[(d+64)%128]; rows 64..127 carry -sin[0:64], rows 0..63 +sin[64:128]
    nsin = np.empty_like(sin)
    nsin[0:64, :] = sin[64:128, :]
    nsin[64:128, :] = -sin[0:64, :]

    kp = np.arange(128)[:, None]
    qf = np.arange(128)[None, :]
    tri = (kp <= qf).astype(ml_dtypes.bfloat16)

    return {
        "cosdt": np.ascontiguousarray(cos),
        "nsindt": np.ascontiguousarray(nsin),
        "trid": tri,
        "onescol": np.ones((128, 1), dtype=ml_dtypes.bfloat16),
    }


_NC_CACHE = None


def _get_nc():
    global _NC_CACHE
    if _NC_CACHE is None:
        _NC_CACHE = build_nc()
    return _NC_CACHE


def kernel(x, Wq, Wk, Wv, Wo, _trace=False):
    x = np.asarray(x, dtype=np.float32)
    Wq = np.asarray(Wq, dtype=np.float32)
    Wk = np.asarray(Wk, dtype=np.float32)
    Wv = np.asarray(Wv, dtype=np.float32)
    Wo = np.asarray(Wo, dtype=np.float32)

    nc = _get_nc()
    consts = _host_consts()

    bf = ml_dtypes.bfloat16
    # x pre-tiled per batch: xt[tb][p][ct*TB + t] = x[b, tb*TB + t, ct*128 + p]
    xts = [
        np.ascontiguousarray(
            x[b].astype(bf).reshape(NTB, TB, NCT, 128).transpose(0, 3, 2, 1).reshape(NTB, 128, NCT * TB)
        )
        for b in range(B)
    ]

    def _tile_w(A, w):  # A: [dout, C] -> [128, NCT*w], w = dout per group
        # out[p, ct*w + j] = A[j, ct*128 + p]
        return np.ascontiguousarray(A.T.reshape(NCT, 128, w).transpose(1, 0, 2).reshape(128, NCT * w)).astype(bf)

    wqs = [_tile_w(Wq[1024 * g : 1024 * (g + 1), :], 1024) for g in range(2)]
    wks = [_tile_w(Wk[512 * g : 512 * (g + 1), :], 512) for g in range(2)]
    wvs = [_tile_w(Wv[512 * g : 512 * (g + 1), :], 512) for g in range(2)]
    # c2-major wo: wo[p, c2*2048 + jh*256 + jo] = Wo[c2*256 + jo, 1024g + jh*128 + p]
    wos = [
        np.ascontiguousarray(
            Wo[:, 1024 * g : 1024 * (g + 1)]
            .T.reshape(NQH, 128, 8, 256)
            .transpose(1, 2, 0, 3)
            .reshape(128, 8 * 2048)
        ).astype(bf)
        for g in range(2)
    ]

    in_maps = []
    for c in range(8):
        b, g = c // 2, c % 2
        im = {
            "xt": xts[b],
            "wq": wqs[g],
            "wk": wks[g],
            "wv": wvs[g],
            "wo": wos[g],
        }
        im.update(consts)
        in_maps.append(im)

    res = run_bass_kernel_spmd(nc, in_maps, core_ids=list(range(8)), trace=_trace)

    y = np.empty((B, T, C), dtype=np.float32)
    for b in range(B):
        y[b] = (res.results[2 * b]["yT"] + res.results[2 * b + 1]["yT"]).T
    if _trace:
        return y, res
    return y
